# revision 13
# baseline (speedup 1.0000x reference)
"""Trainium2 Bass kernel for nn_Coordinator (multi-agent GRU coordinator).

Layout strategy: pure data-parallel over batch B across 8 cores. On-chip
compute is feature-major ([feature_partition, batch_free]); the host
pre-transposes inputs and post-transposes outputs. fp16 operands with fp32
PSUM accumulation; activations on ScalarE; GRU glue on DVE/GPSIMD.
"""

import sys

sys.path.insert(0, "/opt/trn_rl_repo")

import numpy as np

B, N, P, A, R = 16384, 8, 256, 16, 128
D = P - A  # 240
NCORES = 8

_CACHE = {}


def _build(BL, ncores=NCORES):
    """Build the per-core Bass program for local batch size BL."""
    import concourse.bacc as bacc
    import concourse.tile as tile
    import concourse.mybir as mybir

    F16 = mybir.dt.float16
    F32 = mybir.dt.float32
    AF = mybir.ActivationFunctionType
    OP = mybir.AluOpType

    BH = BL // 2          # phase-1 half-batch
    NT = max(1, BL // 1024)   # phase-2 N-tiles of <=1024
    PNT = BL // NT        # phase-2 moving tile size
    assert BL % 2 == 0

    nc = bacc.Bacc("TRN2", target_bir_lowering=False, debug=False,
                   num_devices=ncores)

    dt = nc.dram_tensor
    # ---- inputs (activations, feature-major, fp16) ----
    commT = dt("commT", (N, 2, 128, BL), F16, kind="ExternalInput")
    plansT = dt("plansT", (N, 2, 128, BL), F16, kind="ExternalInput")
    hidT = dt("hidT", (N, 2, 128, BL), F16, kind="ExternalInput")   # padded 240->256
    ghT = dt("ghT", (N, 2, 128, BL), F16, kind="ExternalInput")     # [i, dir, r, b]
    alT = dt("alT", (N, A, BL), F16, kind="ExternalInput")          # + q_b folded

    # ---- weights (fp16, pre-transposed [K-chunk, p, M]) ----
    wihf = dt("wihf", (128, 2, 3 * R), F16, kind="ExternalInput")
    wihr = dt("wihr", (128, 2, 3 * R), F16, kind="ExternalInput")
    whhf = dt("whhf", (128, 3 * R), F16, kind="ExternalInput")
    whhr = dt("whhr", (128, 3 * R), F16, kind="ExternalInput")
    w1 = dt("w1", (128, 2, R), F16, kind="ExternalInput")
    dw2 = dt("dw2", (128, 1), F16, kind="ExternalInput")
    cwih = dt("cwih", (N, 128, 2, 3 * D), F16, kind="ExternalInput")  # rows padded
    cwhh = dt("cwhh", (N, 128, 2, 3 * D), F16, kind="ExternalInput")
    qw = dt("qw", (N, 128, 2, A), F16, kind="ExternalInput")
    identD = dt("identD", (128, 128), F16, kind="ExternalInput")

    # ---- biases (fp32) ----
    # phase1: per dir: rz summed bias [128,2], bhh_n [128,1], bih_n [128,1]
    brzf = dt("brzf", (128, 2), F32, kind="ExternalInput")
    brzr = dt("brzr", (128, 2), F32, kind="ExternalInput")
    bhnf = dt("bhnf", (128, 2), F32, kind="ExternalInput")  # col0=bhh_n col1=bih_n
    bhnr = dt("bhnr", (128, 2), F32, kind="ExternalInput")
    b1 = dt("b1", (128, 1), F32, kind="ExternalInput")
    gb3 = dt("gb3", (1, 2), F32, kind="ExternalInput")      # [gb, -gb]
    # phase2 biases: rz summed [i, 128, 2(chunk), 2(gate)], n separate
    cbrz = dt("cbrz", (N, 128, 2, 2), F32, kind="ExternalInput")
    cbhn = dt("cbhn", (N, 128, 2, 2), F32, kind="ExternalInput")  # [..,0]=bhh_n [..,1]=bih_n

    # ---- outputs ----
    hxsT = dt("hxsT", (N, 2, 128, BL), F16, kind="ExternalOutput")
    coordT = dt("coordT", (N, N, 2, BL), F16, kind="ExternalOutput")
    qT = dt("qT", (N, A, BL), F16, kind="ExternalOutput")
    invT = dt("invT", (N, A, BL), F16, kind="ExternalOutput")

    # ---- staging ----
    ysf = dt("ysf", (N, N, 128, BL), F16)   # fwd outputs, internal
    m_dram = dt("m_dram", (N, N, BL), F16)  # blind masks, internal
    m2_dram = dt("m2_dram", (N, N, BL), F16)  # inverse masks -1-m, internal

    with tile.TileContext(nc) as tc:
        with (
            tc.tile_pool(name="wp", bufs=1) as wp,
            tc.tile_pool(name="state", bufs=1) as st,
            tc.tile_pool(name="stream", bufs=2) as sm,
            tc.tile_pool(name="gi", bufs=2) as gp,
            tc.tile_pool(name="work", bufs=2) as wk,
            tc.tile_pool(name="workA", bufs=8) as wkA,
            tc.tile_pool(name="work1", bufs=1) as wk1,
            tc.tile_pool(name="ps", bufs=4, space="PSUM") as pp,
        ):
            MMN = 512

            def mm(ps, w, x, start, stop):
                n = ps.shape[-1]
                for o in range(0, n, MMN):
                    e = min(o + MMN, n)
                    nc.tensor.matmul(ps[..., o:e], w, x[..., o:e],
                                     start=start, stop=stop)

            # ---------- load weights ----------
            wihf_t = wp.tile([128, 2, 3 * R], F16, name="wihf_t", tag="wihf_t")
            nc.sync.dma_start(wihf_t[:], wihf[:])
            wihr_t = wp.tile([128, 2, 3 * R], F16, name="wihr_t", tag="wihr_t")
            nc.sync.dma_start(wihr_t[:], wihr[:])
            whhf_t = wp.tile([128, 3 * R], F16, name="whhf_t", tag="whhf_t")
            nc.sync.dma_start(whhf_t[:], whhf[:])
            whhr_t = wp.tile([128, 3 * R], F16, name="whhr_t", tag="whhr_t")
            nc.sync.dma_start(whhr_t[:], whhr[:])
            w1_t = wp.tile([128, 2, R], F16, name="w1_t", tag="w1_t")
            nc.sync.dma_start(w1_t[:], w1[:])
            dw2_t = wp.tile([128, 1], F16, name="dw2_t", tag="dw2_t")
            nc.sync.dma_start(dw2_t[:], dw2[:])
            ident_t = wp.tile([128, 128], F16, name="ident_t", tag="ident_t")
            nc.sync.dma_start(ident_t[:], identD[:])
            qw_t = wp.tile([128, N, 2, A], F16, name="qw_t", tag="qw_t")
            nc.sync.dma_start(qw_t[:], qw.ap().rearrange("i p k m -> p i k m"))
            brzf_t = wp.tile([128, 2], F32, name="brzf_t", tag="brzf_t")
            nc.sync.dma_start(brzf_t[:], brzf[:])
            brzr_t = wp.tile([128, 2], F32, name="brzr_t", tag="brzr_t")
            nc.sync.dma_start(brzr_t[:], brzr[:])
            bhnf_t = wp.tile([128, 2], F32, name="bhnf_t", tag="bhnf_t")
            nc.sync.dma_start(bhnf_t[:], bhnf[:])
            bhnr_t = wp.tile([128, 2], F32, name="bhnr_t", tag="bhnr_t")
            nc.sync.dma_start(bhnr_t[:], bhnr[:])
            b1_t = wp.tile([128, 1], F32, name="b1_t", tag="b1_t")
            nc.sync.dma_start(b1_t[:], b1[:])
            gb3_t = wp.tile([1, 2], F32, name="gb3_t", tag="gb3_t")
            nc.sync.dma_start(gb3_t[:], gb3[:])
            cbrz_t = wp.tile([128, N, 2, 2], F32, name="cbrz_t", tag="cbrz_t")
            nc.sync.dma_start(cbrz_t[:], cbrz.ap().rearrange("i p k g -> p i k g"))
            cbhn_t = wp.tile([128, N, 2, 2], F32, name="cbhn_t", tag="cbhn_t")
            nc.sync.dma_start(cbhn_t[:], cbhn.ap().rearrange("i p k g -> p i k g"))

            # ---------- phase 1 ----------
            h1p = [st.tile([128, BH], F16, name=f"h1p{i}", tag=f"h1p{i}")
                   for i in range(N)]

            def proj_p1(x_t, wih_t, name):
                """project [128,2,BH] input through wih -> SBUF f16 [128,3,BH]."""
                gi = gp.tile([128, 3, BH], F16, name=f"gi_{name}", tag=f"gi_{name[0]}")
                for g in range(3):
                    ps = pp.tile([128, BH], F32, name=f"psp_{name}{g}", tag="ps")
                    for k in range(2):
                        mm(ps[:], wih_t[:, k, g * 128:(g + 1) * 128],
                           x_t[:, k, :], start=(k == 0), stop=(k == 1))
                    if g % 2 == 0:
                        nc.vector.tensor_copy(gi[:, g, :], ps[:])
                    else:
                        nc.scalar.copy(gi[:, g, :], ps[:])
                return gi

            def step_p1(h, gi, whh_t, brz_t, bhn_t):
                """one GRU cell update in place on h [128,BH]."""
                psr = pp.tile([128, BH], F32, name="psr", tag="ps")
                psz = pp.tile([128, BH], F32, name="psz", tag="ps")
                psn = pp.tile([128, BH], F32, name="psn", tag="ps")
                mm(psr[:], whh_t[:, 0 * 128:1 * 128], h[:], True, False)
                mm(psr[:], ident_t[:], gi[:, 0, :], False, True)
                mm(psz[:], whh_t[:, 1 * 128:2 * 128], h[:], True, False)
                mm(psz[:], ident_t[:], gi[:, 1, :], False, True)
                mm(psn[:], whh_t[:, 2 * 128:3 * 128], h[:], True, True)
                r = wkA.tile([128, BH], F16, name="r1", tag="wA")
                z = wkA.tile([128, BH], F16, name="z1", tag="wA")
                nc.scalar.activation(r[:], psr[:], AF.Sigmoid, bias=brz_t[:, 0:1])
                nc.scalar.activation(z[:], psz[:], AF.Sigmoid, bias=brz_t[:, 1:2])
                tmp = wkA.tile([128, BH], F16, name="tmp1", tag="wA")
                nc.vector.scalar_tensor_tensor(tmp[:], psn[:], bhn_t[:, 0:1], r[:],
                                               op0=OP.add, op1=OP.mult)
                nc.vector.tensor_tensor(tmp[:], tmp[:], gi[:, 2, :], op=OP.add)
                n = wkA.tile([128, BH], F16, name="n1", tag="wA")
                nc.scalar.activation(n[:], tmp[:], AF.Tanh, bias=bhn_t[:, 1:2])
                d = wkA.tile([128, BH], F16, name="d1", tag="wA")
                nc.gpsimd.tensor_tensor(d[:], h[:], n[:], op=OP.subtract)
                nc.gpsimd.tensor_tensor(d[:], z[:], d[:], op=OP.mult)
                nc.vector.tensor_tensor(h[:], n[:], d[:], op=OP.add)

            for half in range(2):
                bsl = slice(half * BH, (half + 1) * BH)
                # ---- pass A: forward ----
                for i in range(N):
                    nc.sync.dma_start(h1p[i][:], ghT[i, 0, :, bsl])
                for s in range(N):
                    xc = sm.tile([128, 2, BH], F16, name="xcA", tag="xcA")
                    nc.sync.dma_start(xc[:], commT[s, :, :, bsl].rearrange("k p b -> p k b"))
                    xp = sm.tile([128, 2, BH], F16, name="xpA", tag="xpA")
                    nc.sync.dma_start(xp[:], plansT[s, :, :, bsl].rearrange("k p b -> p k b"))
                    gic = proj_p1(xc, wihf_t, "cA")
                    gip = proj_p1(xp, wihf_t, "pA")
                    for i in range(N):
                        step_p1(h1p[i], gip if i == s else gic, whhf_t, brzf_t, bhnf_t)
                        nc.sync.dma_start(ysf[i, s, :, bsl], h1p[i][:])
                for i in range(N):
                    nc.sync.dma_start(hxsT[i, 0, :, bsl], h1p[i][:])
                # ---- pass B: reverse + MLP ----
                for i in range(N):
                    nc.sync.dma_start(h1p[i][:], ghT[i, 1, :, bsl])
                for s in range(N):
                    t = N - 1 - s
                    xc = sm.tile([128, 2, BH], F16, name="xcB", tag="xcA")
                    nc.sync.dma_start(xc[:], commT[t, :, :, bsl].rearrange("k p b -> p k b"))
                    xp = sm.tile([128, 2, BH], F16, name="xpB", tag="xpA")
                    nc.sync.dma_start(xp[:], plansT[t, :, :, bsl].rearrange("k p b -> p k b"))
                    gic = proj_p1(xc, wihr_t, "cB")
                    gip = proj_p1(xp, wihr_t, "pB")
                    for i in range(N):
                        step_p1(h1p[i], gip if i == t else gic, whhr_t, brzr_t, bhnr_t)
                        # MLP on (i, t): scores = [ysf[i,t]; h1p[i]]
                        yf = sm.tile([128, BH], F16, name="yf", tag="yf")
                        nc.sync.dma_start(yf[:], ysf[i, t, :, bsl])
                        ps1 = pp.tile([128, BH], F32, name="ps1m", tag="ps")
                        mm(ps1[:], w1_t[:, 0, :], yf[:], True, False)
                        mm(ps1[:], w1_t[:, 1, :], h1p[i][:], False, True)
                        hh1 = wkA.tile([128, BH], F16, name="hh1", tag="wA")
                        nc.scalar.activation(hh1[:], ps1[:], AF.Relu, bias=b1_t[:])
                        psg = pp.tile([1, BH], F32, name="psg", tag="ps")
                        mm(psg[:], dw2_t[:], hh1[:], True, True)
                        # blind mask: gap + gb > 0  <=>  gap > -gb
                        mseg = wk1.tile([1, BH], F16, name="mseg", tag="mseg")
                        nc.vector.tensor_scalar(mseg[:], psg[:], gb3_t[0:1, 1:2],
                                                None, op0=OP.is_gt)
                        nc.sync.dma_start(m_dram[i, t, bsl], mseg[:])
                        m2seg = wk1.tile([1, BH], F16, name="m2seg", tag="m2seg")
                        nc.vector.tensor_scalar(m2seg[:], mseg[:], -1.0, 1.0,
                                                op0=OP.mult, op1=OP.subtract)
                        nc.sync.dma_start(m2_dram[i, t, bsl], m2seg[:])
                        c1 = wk1.tile([1, BH], F16, name="c1", tag="c1")
                        nc.scalar.activation(c1[:], psg[:], AF.Sigmoid, bias=gb3_t[0:1, 0:1])
                        nc.sync.dma_start(coordT[i, t, 1, bsl], c1[:])
                        c0 = wk1.tile([1, BH], F16, name="c0", tag="c0")
                        nc.scalar.activation(c0[:], psg[:], AF.Sigmoid,
                                             bias=gb3_t[0:1, 1:2], scale=-1.0)
                        nc.sync.dma_start(coordT[i, t, 0, bsl], c0[:])
                for i in range(N):
                    nc.sync.dma_start(hxsT[i, 1, :, bsl], h1p[i][:])

            # ---------- phase 2 ----------
            hq = st.tile([128, 2, BL], F16, name="hq", tag="hq")
            hv = st.tile([128, 2, BL], F16, name="hv", tag="hv")
            SUBS = [(0, 128), (1, 112)]  # chunk, valid rows

            for i in range(N):
                cwih_t = wk1.tile([128, 2, 3 * D], F16, name="cwih_t", tag="cwih_t")
                nc.sync.dma_start(cwih_t[:], cwih[i])
                cwhh_t = wk1.tile([128, 2, 3 * D], F16, name="cwhh_t", tag="cwhh_t")
                nc.sync.dma_start(cwhh_t[:], cwhh[i])
                nc.sync.dma_start(hq[:], hidT[i].rearrange("k p b -> p k b"))
                nc.sync.dma_start(hv[:], hidT[i].rearrange("k p b -> p k b"))
                for s in range(N - 1):
                    j = s if s < i else s + 1
                    mrow = wk1.tile([1, BL], F16, name="mrow", tag="mrow")
                    nc.sync.dma_start(mrow[:], m_dram[i, j, :])
                    m2row = wk1.tile([1, BL], F16, name="m2row", tag="m2row")
                    nc.sync.dma_start(m2row[:], m2_dram[i, j, :])
                    mb = wk1.tile([128, BL], F16, name="mb", tag="mb")
                    nc.gpsimd.partition_broadcast(mb[:], mrow[:])
                    m2b = wk1.tile([128, BL], F16, name="m2b", tag="m2b")
                    nc.gpsimd.partition_broadcast(m2b[:], m2row[:])
                    xs = []
                    for nt in range(NT):
                        nsl = slice(nt * PNT, (nt + 1) * PNT)
                        cp = sm.tile([128, 2, PNT], F16, name="cp2", tag="cp2")
                        nc.sync.dma_start(cp[:], commT[j, :, :, nsl].rearrange("k p b -> p k b"))
                        xq = wk.tile([128, 2, PNT], F16, name="xq", tag="xq")
                        xv = wk.tile([128, 2, PNT], F16, name="xv", tag="xv")
                        for k in range(2):
                            nc.gpsimd.tensor_tensor(xq[:, k, :], cp[:, k, :], mb[:, nsl], op=OP.mult)
                            nc.gpsimd.tensor_tensor(xv[:, k, :], cp[:, k, :], m2b[:, nsl], op=OP.mult)
                        xs.append((xq, xv))
                    for ci, h in ((0, hq), (1, hv)):
                        for nt in range(NT):
                            nsl = slice(nt * PNT, (nt + 1) * PNT)
                            x = xs[nt][ci]
                            rzt = wk1.tile([128, 2, 2, PNT], F16, name="rzt", tag="rzt")
                            for g in range(2):       # r, z
                                for c, M in SUBS:
                                    cols = slice(g * D + c * 128, g * D + c * 128 + M)
                                    ps = pp.tile([M, PNT], F32, name=f"p2{g}{c}", tag="ps")
                                    mm(ps[:], cwih_t[:, 0, cols], x[:, 0, :], True, False)
                                    mm(ps[:], cwih_t[:, 1, cols], x[:, 1, :], False, False)
                                    mm(ps[:], cwhh_t[:, 0, cols], h[:, 0, nsl], False, False)
                                    mm(ps[:], cwhh_t[:, 1, cols], h[:, 1, nsl], False, True)
                                    nc.scalar.activation(rzt[0:M, g, c, :], ps[:], AF.Sigmoid,
                                                         bias=cbrz_t[0:M, i, c, g:g + 1])
                            nt_n = wk1.tile([128, 2, PNT], F16, name="nt_n", tag="nt_n")
                            for c, M in SUBS:
                                cols = slice(2 * D + c * 128, 2 * D + c * 128 + M)
                                psh = pp.tile([M, PNT], F32, name=f"p2h{c}", tag="ps")
                                mm(psh[:], cwhh_t[:, 0, cols], h[:, 0, nsl], True, False)
                                mm(psh[:], cwhh_t[:, 1, cols], h[:, 1, nsl], False, True)
                                psi = pp.tile([M, PNT], F32, name=f"p2i{c}", tag="ps")
                                mm(psi[:], cwih_t[:, 0, cols], x[:, 0, :], True, False)
                                mm(psi[:], cwih_t[:, 1, cols], x[:, 1, :], False, True)
                                tmp = wk.tile([128, PNT], F16, name="tmp2", tag="tmp2")
                                nc.vector.scalar_tensor_tensor(tmp[0:M, :], psh[:], cbhn_t[0:M, i, c, 0:1],
                                                               rzt[0:M, 0, c, :], op0=OP.add, op1=OP.mult)
                                nc.vector.tensor_tensor(tmp[0:M, :], tmp[0:M, :], psi[:], op=OP.add)
                                nc.scalar.activation(nt_n[0:M, c, :], tmp[0:M, :], AF.Tanh,
                                                     bias=cbhn_t[0:M, i, c, 1:2])
                            # h' = n + z*(h-n)
                            for c, M in SUBS:
                                d2 = wk.tile([128, PNT], F16, name="d2", tag="d2")
                                nc.vector.tensor_tensor(d2[0:M, :], h[0:M, c, nsl], nt_n[0:M, c, :], op=OP.subtract)
                                nc.vector.tensor_tensor(d2[0:M, :], rzt[0:M, 1, c, :], d2[0:M, :], op=OP.mult)
                                nc.vector.tensor_tensor(h[0:M, c, nsl], nt_n[0:M, c, :], d2[0:M, :], op=OP.add)
                # q head
                al = wk1.tile([A, BL], F16, name="al2", tag="al2")
                nc.sync.dma_start(al[:], alT[i])
                for h, outD in ((hq, qT), (hv, invT)):
                    qo = wk1.tile([A, BL], F16, name="qo", tag="qo")
                    for nt in range(NT):
                        nsl = slice(nt * PNT, (nt + 1) * PNT)
                        psq = pp.tile([A, PNT], F32, name="psq", tag="ps")
                        mm(psq[:], qw_t[:, i, 0, :], h[:, 0, nsl], True, False)
                        mm(psq[:], qw_t[:, i, 1, :], h[:, 1, nsl], False, True)
                        nc.vector.tensor_tensor(qo[:, nsl], psq[:], al[:, nsl], op=OP.add)
                    nc.sync.dma_start(outD[i], qo[:])

    nc.compile()
    return nc


def _prep(inputs, BL, core):
    """Build the per-core input map (host-side reformatting)."""
    f16 = np.float16
    f32 = np.float32
    b0 = core * BL
    bs = slice(b0, b0 + BL)

    def t3(x):  # [BL, N, F] -> [N, 2, 128, BL]
        v = np.ascontiguousarray(np.transpose(x[bs], (1, 2, 0)))  # [N, F, BL]
        F = v.shape[1]
        if F < 256:
            v = np.concatenate([v, np.zeros((N, 256 - F, BL), v.dtype)], axis=1)
        return np.ascontiguousarray(v.reshape(N, 2, 128, BL).astype(f16))

    im = {
        "commT": t3(inputs["comm_plans"]),
        "plansT": t3(inputs["plans"]),
        "hidT": t3(inputs["hiddens"]),
        "ghT": np.ascontiguousarray(
            np.transpose(inputs["glob_hiddens"][:, :, bs, :], (0, 1, 3, 2))).astype(f16),
        "alT": np.ascontiguousarray(
            (np.transpose(inputs["action_logits"][bs], (1, 2, 0)).astype(f32)
             + np.asarray(inputs["q_b"], f32)[:, :, None]).astype(f16)),
    }
    return im


def _prep_weights(inputs):
    f16 = np.float16
    f32 = np.float32

    def wT(w):  # [M, K] -> [128, 2, M] chunks of K
        v = np.ascontiguousarray(w.T.astype(f16))  # [K, M]
        K, M = v.shape
        if K < 256:
            v = np.concatenate([v, np.zeros((256 - K, M), f16)], 0)
        return np.ascontiguousarray(v.reshape(2, 128, M).transpose(1, 0, 2))

    def wT1(w):  # [M, 128] -> [128, M]
        return np.ascontiguousarray(w.T.astype(f16))

    def cw(w):  # [N, M, K(240)] -> [N, 128, 2, M]
        out = np.zeros((N, 256, w.shape[1]), f16)
        out[:, :w.shape[2], :] = np.transpose(w, (0, 2, 1)).astype(f16)
        return np.ascontiguousarray(out.reshape(N, 2, 128, -1).transpose(0, 2, 1, 3))

    def bsplit(b):  # [3D] -> [128, 2, 3] padded per gate/chunk
        out = np.zeros((3, 256), f32)
        for g in range(3):
            out[g, :D] = b[g * D:(g + 1) * D]
        return out.reshape(3, 2, 128).transpose(2, 1, 0)  # [128, 2, 3]

    bihf, bhhf = np.asarray(inputs["b_ih_f"], f32), np.asarray(inputs["b_hh_f"], f32)
    bihr, bhhr = np.asarray(inputs["b_ih_r"], f32), np.asarray(inputs["b_hh_r"], f32)
    b2 = np.asarray(inputs["bc_b2"], f32)
    gb = float(b2[1] - b2[0])

    wm = {
        "wihf": wT(np.asarray(inputs["w_ih_f"])),
        "wihr": wT(np.asarray(inputs["w_ih_r"])),
        "whhf": wT1(np.asarray(inputs["w_hh_f"])).reshape(128, 384),
        "whhr": wT1(np.asarray(inputs["w_hh_r"])).reshape(128, 384),
        "w1": wT(np.asarray(inputs["bc_w1"])),
        "dw2": np.ascontiguousarray(
            (np.asarray(inputs["bc_w2"], f32)[1] - np.asarray(inputs["bc_w2"], f32)[0])
            .astype(f16)[:, None]),
        "identD": np.eye(128, dtype=f16),
        "brzf": np.stack([(bihf + bhhf)[0:R], (bihf + bhhf)[R:2 * R]], 1).astype(f32),
        "brzr": np.stack([(bihr + bhhr)[0:R], (bihr + bhhr)[R:2 * R]], 1).astype(f32),
        "bhnf": np.stack([bhhf[2 * R:], bihf[2 * R:]], 1).astype(f32),
        "bhnr": np.stack([bhhr[2 * R:], bihr[2 * R:]], 1).astype(f32),
        "b1": np.asarray(inputs["bc_b1"], f32)[:, None],
        "gb3": np.array([[gb, -gb]], f32),
    }
    cbih = np.asarray(inputs["cb_ih"], f32)
    cbhh = np.asarray(inputs["cb_hh"], f32)
    cbrz = np.zeros((N, 128, 2, 2), f32)
    cbhn = np.zeros((N, 128, 2, 2), f32)
    for i in range(N):
        srz = bsplit(cbih[i] + cbhh[i])      # [128, 2, 3]
        cbrz[i] = srz[:, :, 0:2]
        cbhn[i, :, :, 0] = bsplit(cbhh[i])[:, :, 2]
        cbhn[i, :, :, 1] = bsplit(cbih[i])[:, :, 2]
    wm["cbrz"] = cbrz
    wm["cbhn"] = cbhn
    wm["cwih"] = cw(np.asarray(inputs["cw_ih"]))
    wm["cwhh"] = cw(np.asarray(inputs["cw_hh"]))
    qwp = np.zeros((N, 256, A), np.float16)
    qwp[:, :D, :] = np.transpose(np.asarray(inputs["q_w"]), (0, 2, 1)).astype(f16)
    wm["qw"] = np.ascontiguousarray(qwp.reshape(N, 2, 128, A).transpose(0, 2, 1, 3))
    return wm


def _run(inputs, BL, ncores, **rkw):
    from concourse.bass_utils import run_bass_kernel_spmd

    key = (BL, ncores)
    if key not in _CACHE:
        _CACHE[key] = _build(BL, ncores)
    nc = _CACHE[key]

    wm = _prep_weights(inputs)
    in_maps = []
    for c in range(ncores):
        im = _prep(inputs, BL, c)
        im.update(wm)
        in_maps.append(im)

    res = run_bass_kernel_spmd(nc, in_maps, core_ids=list(range(ncores)), **rkw)
    _CACHE["last_results"] = res

    f32 = np.float32
    qs, invs, hxs, cms = [], [], [], []
    for c in range(ncores):
        r = res.results[c]
        qs.append(np.transpose(r["qT"].astype(f32), (2, 0, 1)))        # [BL, N, A]
        invs.append(np.transpose(r["invT"].astype(f32), (2, 0, 1)))
        hxs.append(np.transpose(r["hxsT"].astype(f32), (0, 1, 3, 2)))  # [N,2,BL,R]
        cms.append(np.transpose(r["coordT"].astype(f32), (0, 1, 3, 2)))
    q_values = np.concatenate(qs, 0).astype(f32)
    inv_q_values = np.concatenate(invs, 0).astype(f32)
    glob_rnn_hxs = np.concatenate(hxs, 2).astype(f32)
    coord_masks = np.concatenate(cms, 2).astype(f32)
    if not int(np.asarray(inputs.get("eval_coord", 1))):
        inv_q_values = None
    return q_values, inv_q_values, glob_rnn_hxs, coord_masks


def kernel(**inputs):
    return _run(inputs, B // NCORES, NCORES)
